# revision 9
# baseline (speedup 1.0000x reference)
"""CGRUCell (GRU -> Bahdanau attention -> GRU) fused Trainium2 kernel.

Data-parallel over batch: 256 rows -> 8 NeuronCores x 32 rows.
All heavy matmuls run on the PE array in bf16 with fp32 PSUM accumulation.
The context tensor is pre-transposed on the host so the dominant
cache = context @ wk.T matmul (17.2 GFLOP/core) needs no on-device
transposes; attention (contraction over L) is done on the vector engine
via fused multiply-reduce in the same transposed layout.
"""

import numpy as np
import ml_dtypes

import concourse.bass as bass
import concourse.tile as tile
from concourse import mybir
from concourse import bass2jax

f32 = mybir.dt.float32
bf16 = mybir.dt.bfloat16

B, L, H, IN, CTX = 256, 128, 1024, 1024, 2048
G3 = 3 * H
N_CORES = 8
BS = B // N_CORES          # 32 batch rows per core
GB = 4                     # batch rows per group (N = GB*L = 512)
NG = BS // GB              # 8 groups
NW = 512                   # matmul moving free dim / psum bank
KH = H // 128              # 8 k-chunks of H
KC = CTX // 128            # 16 k-chunks of CTX

_BF = ml_dtypes.bfloat16


def _split_multi_waits(nc, max_waits=1):
    """This container's walrus build cannot encode >1 sem waits on
    NO_STRUCT ops (the TileContext exit drain).  Split extra waits onto
    preceding single-wait drains on the same engine."""
    n_new = 0
    for fn in nc.m.functions:
        for blk in fn.blocks:
            new_list = []
            for inst in blk.instructions:
                si = inst.sync_info
                if si is not None and len(si.on_wait) > max_waits:
                    waits = list(si.on_wait)
                    for w in waits[max_waits:]:
                        nm = f"I-waitsplit-{n_new}"
                        n_new += 1
                        d = mybir.InstDrain(
                            name=nm, engine=inst.engine,
                            sync_info=mybir.SyncInfo(on_wait=[w], on_update=[]),
                            ins=[], outs=[],
                        )
                        nc.inst_map[nm] = d
                        new_list.append(d)
                    inst.sync_info = mybir.SyncInfo(
                        on_wait=waits[:max_waits], on_update=list(si.on_update))
                new_list.append(inst)
            blk.instructions = new_list
    return n_new


def _gru_pointwise(nc, pool, gi, gh, h_prev, name):
    """h' = (1-z)*n + z*h for torch GRUCell gate layout [r|z|n].
    gi/gh: [BS, 3H] APs (SBUF or PSUM), h_prev: [BS, H] AP. Returns [BS,H] tile.
    Temp tags are shared between both GRU phases (they never overlap)."""
    t_rz = pool.tile([BS, 2 * H], f32, tag="gw_trz", name="t_rz")
    nc.vector.tensor_add(t_rz[:], gi[:, 0:2 * H], gh[:, 0:2 * H])
    rz = pool.tile([BS, 2 * H], f32, tag="gw_rz", name="rz")
    nc.scalar.activation(rz[:], t_rz[:], mybir.ActivationFunctionType.Sigmoid)
    t_n = pool.tile([BS, H], f32, tag="gw_tn", name="t_n")
    nc.vector.tensor_mul(t_n[:], rz[:, 0:H], gh[:, 2 * H:3 * H])
    nc.vector.tensor_add(t_n[:], gi[:, 2 * H:3 * H], t_n[:])
    n_t = pool.tile([BS, H], f32, tag="gw_n", name="n_t")
    nc.scalar.activation(n_t[:], t_n[:], mybir.ActivationFunctionType.Tanh)
    t_s = pool.tile([BS, H], f32, tag="gw_ts", name="t_s")
    nc.vector.tensor_sub(t_s[:], h_prev[:], n_t[:])
    nc.vector.tensor_mul(t_s[:], rz[:, H:2 * H], t_s[:])
    h_out = pool.tile([BS, H], f32, tag=f"{name}_h", name=f"{name}_h")
    nc.vector.tensor_add(h_out[:], n_t[:], t_s[:])
    return h_out


def build_nc(use_b1=False, use_b2=False, use_bqk=False):
    nc = bass.Bass("TRN2", target_bir_lowering=False, debug=False)

    dram = {}
    def din(name, shape, dt):
        dram[name] = nc.dram_tensor(name, shape, dt, kind="ExternalInput").ap()
        return dram[name]

    xT = din("xT", [IN, BS], bf16)
    hT = din("hT", [H, BS], bf16)
    h_nat = din("h_nat", [BS, H], f32)
    ctxT = din("ctxT", [NG, CTX, GB * L], bf16)
    w_ih1T = din("w_ih1T", [IN, G3], bf16)
    w_hh1T = din("w_hh1T", [H, G3], bf16)
    wqT = din("wqT", [H, H], bf16)
    wkT = din("wkT", [CTX, H], bf16)
    w_ih2T = din("w_ih2T", [CTX, G3], bf16)
    w_hh2T = din("w_hh2T", [H, G3], bf16)
    wv8 = din("wv8", [128, KH], bf16)
    ones1 = din("ones1", [1, 128], f32)
    ident = din("ident", [128, 128], f32)
    if use_bqk:
        bqk8 = din("bqk8", [128, KH], f32)
    if use_b1:
        b_ih1 = din("b_ih1", [G3], f32)
        b_hh1 = din("b_hh1", [G3], f32)
    if use_b2:
        b_ih2 = din("b_ih2", [G3], f32)
        b_hh2 = din("b_hh2", [G3], f32)

    out_h2 = nc.dram_tensor("hidden2", [BS, H], f32, kind="ExternalOutput").ap()
    out_at = nc.dram_tensor("attn", [BS, CTX], f32, kind="ExternalOutput").ap()

    Tanh = mybir.ActivationFunctionType.Tanh
    Exp = mybir.ActivationFunctionType.Exp
    Alu = mybir.AluOpType

    with tile.TileContext(nc) as tc:
        with (
            tc.tile_pool(name="const", bufs=1) as constp,
            tc.tile_pool(name="wstream", bufs=8) as wsp,
            tc.tile_pool(name="slab", bufs=1) as slabp,
            tc.tile_pool(name="ctx", bufs=24) as ctxp,
            tc.tile_pool(name="tanh", bufs=10) as tanhp,
            tc.tile_pool(name="small", bufs=2) as smallp,
        ):
            # ---- constants / persistent tiles ----
            ident_sb = constp.tile([128, 128], f32, tag="ident")
            nc.sync.dma_start(ident_sb[:], ident[:])
            ones_sb = constp.tile([1, 128], f32, tag="ones")
            nc.sync.dma_start(ones_sb[:], ones1[:])
            wv_sb = constp.tile([128, KH], bf16, tag="wv8")
            nc.sync.dma_start(wv_sb[:], wv8[:])
            if use_bqk:
                bqk_sb = constp.tile([128, KH], f32, tag="bqk8")
                nc.sync.dma_start(bqk_sb[:], bqk8[:])

            xT_sb = constp.tile([128, IN // 128, BS], bf16, tag="xT")
            nc.sync.dma_start(xT_sb[:], xT.rearrange("(k p) b -> p k b", p=128))
            hT_sb = constp.tile([128, KH, BS], bf16, tag="hT")
            nc.sync.dma_start(hT_sb[:], hT.rearrange("(k p) b -> p k b", p=128))
            h_nat_sb = constp.tile([BS, H], f32, tag="h_nat")
            nc.sync.dma_start(h_nat_sb[:], h_nat[:])

            wkT_sb = []
            for k in range(KC):
                t = constp.tile([128, H], bf16, tag=f"wkT{k}")
                nc.sync.dma_start(t[:], wkT[k * 128:(k + 1) * 128, :])
                wkT_sb.append(t)

            if use_b1:
                bih1_bc = constp.tile([BS, G3], f32, tag="bih1")
                nc.sync.dma_start(bih1_bc[:], b_ih1.partition_broadcast(BS))
                bhh1_bc = constp.tile([BS, G3], f32, tag="bhh1")
                nc.sync.dma_start(bhh1_bc[:], b_hh1.partition_broadcast(BS))
            if use_b2:
                bih2_bc = constp.tile([BS, G3], f32, tag="bih2")
                nc.sync.dma_start(bih2_bc[:], b_ih2.partition_broadcast(BS))
                bhh2_bc = constp.tile([BS, G3], f32, tag="bhh2")
                nc.sync.dma_start(bhh2_bc[:], b_hh2.partition_broadcast(BS))

            def stream_mm(psum_ap, lhs_tiles, w_dram, kchunks, name):
                """psum[BS, G] += lhsT_k.T @ w[k*128:+128, :] streamed in NW cols."""
                ncols = w_dram.shape[1]
                for ns in range(ncols // NW):
                    for k in range(kchunks):
                        wt = wsp.tile([128, NW], bf16, tag="w")
                        nc.sync.dma_start(
                            wt[:], w_dram[k * 128:(k + 1) * 128,
                                          ns * NW:(ns + 1) * NW])
                        nc.tensor.matmul(
                            psum_ap[:, ns * NW:(ns + 1) * NW],
                            lhs_tiles(k), wt[:],
                            start=(k == 0), stop=(k == kchunks - 1))

            # ================= phase 1: GRU1 + q =================
            with (
                tc.tile_pool(name="ps1", bufs=1, space="PSUM") as ps1,
                tc.tile_pool(name="tp1", bufs=2, space="PSUM") as tp1,
            ):
                gi1_ps = ps1.tile([BS, G3], f32, tag="gg")
                stream_mm(gi1_ps, lambda k: xT_sb[:, k, :], w_ih1T, IN // 128, "gi1")
                gi1_sb = slabp.tile([BS, G3], f32, tag="giX")
                if use_b1:
                    nc.vector.tensor_add(gi1_sb[:], gi1_ps[:], bih1_bc[:])
                else:
                    nc.vector.tensor_copy(gi1_sb[:], gi1_ps[:])

                gh1_ps = ps1.tile([BS, G3], f32, tag="gg")
                stream_mm(gh1_ps, lambda k: hT_sb[:, k, :], w_hh1T, KH, "gh1")
                gh1_sb = slabp.tile([BS, G3], f32, tag="ghX")
                if use_b1:
                    nc.vector.tensor_add(gh1_sb[:], gh1_ps[:], bhh1_bc[:])
                else:
                    nc.vector.tensor_copy(gh1_sb[:], gh1_ps[:])

                h1 = _gru_pointwise(nc, slabp, gi1_sb, gh1_sb, h_nat_sb, "g1")

                # h1T (bf16) via PE transpose, for q and gh2 matmuls
                h1T = []
                for k in range(KH):
                    tp = tp1.tile([128, BS], f32, tag="tp")
                    nc.tensor.transpose(tp[:], h1[:, k * 128:(k + 1) * 128],
                                        ident_sb[0:BS, 0:BS])
                    t = slabp.tile([128, BS], bf16, tag=f"h1T{k}")
                    nc.vector.tensor_copy(t[:], tp[:])
                    h1T.append(t)

                q_ps = ps1.tile([BS, H], f32, tag="gg")
                stream_mm(q_ps, lambda k: h1T[k][:], wqT, KH, "q")
                q_sb = slabp.tile([BS, H], f32, tag="q")
                nc.scalar.copy(q_sb[:], q_ps[:])

                # qbkT[k] = (q + bq + bk).T chunk, [128, BS] f32
                qbkT = []
                for k in range(KH):
                    tp = tp1.tile([128, BS], f32, tag="tp")
                    nc.tensor.transpose(tp[:], q_sb[:, k * 128:(k + 1) * 128],
                                        ident_sb[0:BS, 0:BS])
                    t = slabp.tile([128, BS], f32, tag=f"qbkT{k}")
                    if use_bqk:
                        nc.vector.tensor_scalar_add(t[:], tp[:], bqk_sb[:, k:k + 1])
                    else:
                        nc.vector.tensor_copy(t[:], tp[:])
                    qbkT.append(t)

            # ============ phase 2: attention scores + weights ============
            attnT_raw = []
            for k in range(KC):
                attnT_raw.append(slabp.tile([128, BS], f32, tag=f"atr{k}", name=f"atr{k}"))
            recip_all = slabp.tile([1, BS], f32, tag="recip")

            with (
                tc.tile_pool(name="psc", bufs=4, space="PSUM") as psc,
                tc.tile_pool(name="psl", bufs=2, space="PSUM") as psl,
                tc.tile_pool(name="psw", bufs=2, space="PSUM") as psw,
            ):
                for g in range(NG):
                    ctx_g = []
                    for k in range(KC):
                        t = ctxp.tile([128, GB * L], bf16, tag="ctx")
                        nc.sync.dma_start(t[:], ctxT[g, k * 128:(k + 1) * 128, :])
                        ctx_g.append(t)

                    # cacheT[h, (b,l)] for the 4 batch rows of this group
                    tanh_g = []
                    for ht in range(KH):
                        cps = psc.tile([128, NW], f32, tag="cache")
                        for k in range(KC):
                            nc.tensor.matmul(
                                cps[:], wkT_sb[k][:, ht * 128:(ht + 1) * 128],
                                ctx_g[k][:], start=(k == 0), stop=(k == KC - 1))
                        tsb = tanhp.tile([128, NW], bf16, tag="tanh")
                        for bb in range(GB):
                            nc.scalar.activation(
                                tsb[:, bb * L:(bb + 1) * L],
                                cps[:, bb * L:(bb + 1) * L],
                                Tanh, bias=qbkT[ht][:, g * GB + bb:g * GB + bb + 1])
                        tanh_g.append(tsb)

                    # logits[1, (b,l)] = wv . tanh
                    lps = psl.tile([1, NW], f32, tag="logit")
                    for ht in range(KH):
                        nc.tensor.matmul(lps[:], wv_sb[:, ht:ht + 1], tanh_g[ht][:],
                                         start=(ht == 0), stop=(ht == KH - 1))

                    # softmax over l per batch row (no max-sub: logits are O(1))
                    exp_sb = smallp.tile([1, NW], f32, tag="exp")
                    sums = smallp.tile([1, GB], f32, tag="sums")
                    for bb in range(GB):
                        nc.scalar.activation(
                            exp_sb[:, bb * L:(bb + 1) * L],
                            lps[:, bb * L:(bb + 1) * L], Exp,
                            accum_out=sums[:, bb:bb + 1])
                    nc.vector.reciprocal(recip_all[:, g * GB:(g + 1) * GB], sums[:])

                    # broadcast raw exp weights across partitions via outer product
                    wps = psw.tile([128, NW], f32, tag="wb")
                    nc.tensor.matmul(wps[:], ones_sb[:], exp_sb[:],
                                     start=True, stop=True)
                    wb_sb = smallp.tile([128, NW], f32, tag="wbsb")
                    nc.scalar.copy(wb_sb[:], wps[:])

                    # attnT_raw[c, b] += sum_l ctxT[c, l] * w[b, l]
                    for k in range(KC):
                        for bb in range(GB):
                            scr = smallp.tile([128, L], bf16, tag="scr")
                            nc.vector.scalar_tensor_tensor(
                                out=scr[:],
                                in0=ctx_g[k][:, bb * L:(bb + 1) * L],
                                scalar=1.0,
                                in1=wb_sb[:, bb * L:(bb + 1) * L],
                                op0=Alu.mult, op1=Alu.mult,
                                accum_out=attnT_raw[k][:, g * GB + bb:g * GB + bb + 1])

            # ============ phase 3: normalize attn + GRU2 + outputs ============
            with (
                tc.tile_pool(name="ps3", bufs=1, space="PSUM") as ps3,
                tc.tile_pool(name="tp3", bufs=2, space="PSUM") as tp3,
            ):
                rb_ps = tp3.tile([128, BS], f32, tag="tp")
                nc.tensor.matmul(rb_ps[:], ones_sb[:], recip_all[:],
                                 start=True, stop=True)
                recip_bc = slabp.tile([128, BS], f32, tag="recipbc")
                nc.vector.tensor_copy(recip_bc[:], rb_ps[:])

                attnT = []
                attnT_b16 = []
                for k in range(KC):
                    t = slabp.tile([128, BS], f32, tag=f"atn{k}")
                    nc.vector.tensor_mul(t[:], attnT_raw[k][:], recip_bc[:])
                    attnT.append(t)
                    tb = slabp.tile([128, BS], bf16, tag=f"atb{k}")
                    nc.vector.tensor_copy(tb[:], t[:])
                    attnT_b16.append(tb)

                gi2_ps = ps3.tile([BS, G3], f32, tag="gg")
                stream_mm(gi2_ps, lambda k: attnT_b16[k][:], w_ih2T, KC, "gi2")
                gi2_sb = slabp.tile([BS, G3], f32, tag="giX")
                if use_b2:
                    nc.vector.tensor_add(gi2_sb[:], gi2_ps[:], bih2_bc[:])
                else:
                    nc.vector.tensor_copy(gi2_sb[:], gi2_ps[:])

                gh2_ps = ps3.tile([BS, G3], f32, tag="gg")
                stream_mm(gh2_ps, lambda k: h1T[k][:], w_hh2T, KH, "gh2")
                gh2_sb = slabp.tile([BS, G3], f32, tag="ghX")
                if use_b2:
                    nc.vector.tensor_add(gh2_sb[:], gh2_ps[:], bhh2_bc[:])
                else:
                    nc.vector.tensor_copy(gh2_sb[:], gh2_ps[:])

                h2 = _gru_pointwise(nc, slabp, gi2_sb, gh2_sb, h1, "g2")
                nc.sync.dma_start(out_h2[:], h2[:])

                # attn natural layout [b, c] for output
                attn_nat = slabp.tile([BS, CTX], f32, tag="attnnat")
                for k in range(KC):
                    tp = tp3.tile([BS, 128], f32, tag="tp", name="tpn")
                    nc.tensor.transpose(tp[:], attnT[k][:], ident_sb[:])
                    nc.scalar.copy(attn_nat[:, k * 128:(k + 1) * 128], tp[:])
                nc.sync.dma_start(out_at[:], attn_nat[:])

    _split_multi_waits(nc)
    return nc


# ---------------- host side ----------------

def _prep_shared(inp):
    """Weight-derived arrays, shared by all cores."""
    def tb(a):
        return np.ascontiguousarray(a.T).astype(_BF)
    out = {
        "w_ih1T": tb(inp["w_ih1"]),
        "w_hh1T": tb(inp["w_hh1"]),
        "wqT": tb(inp["wq"]),
        "wkT": tb(inp["wk"]),
        "w_ih2T": tb(inp["w_ih2"]),
        "w_hh2T": tb(inp["w_hh2"]),
        "wv8": np.ascontiguousarray(
            inp["wv"][0].reshape(KH, 128).T).astype(_BF),
        "ones1": np.ones((1, 128), np.float32),
        "ident": np.eye(128, dtype=np.float32),
    }
    flags = {}
    flags["use_bqk"] = bool(np.any(inp["bq"])) or bool(np.any(inp["bk"]))
    flags["use_b1"] = bool(np.any(inp["b_ih1"])) or bool(np.any(inp["b_hh1"]))
    flags["use_b2"] = bool(np.any(inp["b_ih2"])) or bool(np.any(inp["b_hh2"]))
    if flags["use_bqk"]:
        out["bqk8"] = np.ascontiguousarray(
            (inp["bq"] + inp["bk"]).reshape(KH, 128).T).astype(np.float32)
    if flags["use_b1"]:
        out["b_ih1"] = inp["b_ih1"].astype(np.float32)
        out["b_hh1"] = inp["b_hh1"].astype(np.float32)
    if flags["use_b2"]:
        out["b_ih2"] = inp["b_ih2"].astype(np.float32)
        out["b_hh2"] = inp["b_hh2"].astype(np.float32)
    return out, flags


def _prep_core(inp, c, shared):
    s = slice(c * BS, (c + 1) * BS)
    x = np.asarray(inp["input"][s], np.float32)
    h = np.asarray(inp["hidden"][s], np.float32)
    ctx = np.asarray(inp["context"][s], np.float32)
    m = dict(shared)
    m["xT"] = np.ascontiguousarray(x.T).astype(_BF)
    m["hT"] = np.ascontiguousarray(h.T).astype(_BF)
    m["h_nat"] = np.ascontiguousarray(h)
    # [BS, L, CTX] -> [NG, CTX, GB*L]
    ct = ctx.transpose(0, 2, 1).reshape(NG, GB, CTX, L)
    ct = np.ascontiguousarray(ct.transpose(0, 2, 1, 3)).reshape(NG, CTX, GB * L)
    m["ctxT"] = ct.astype(_BF)
    return m


_RUNNERS = {}


def _make_runner(flags_key):
    import jax
    from jax.experimental.shard_map import shard_map
    from jax.sharding import Mesh, PartitionSpec

    nc = build_nc(*flags_key)
    bass2jax.install_neuronx_cc_hook()

    part_name = nc.partition_id_tensor.name if nc.partition_id_tensor else None
    in_names, out_names, out_avals, zero_shapes = [], [], [], []
    for alloc in nc.m.functions[0].allocations:
        if not isinstance(alloc, mybir.MemoryLocationSet):
            continue
        name = alloc.memorylocations[0].name
        if alloc.kind == "ExternalInput":
            if name != part_name:
                in_names.append(name)
        elif alloc.kind == "ExternalOutput":
            out_names.append(name)
            shape = tuple(alloc.tensor_shape)
            dtype = mybir.dt.np(alloc.dtype)
            out_avals.append(jax.core.ShapedArray(shape, dtype))
            zero_shapes.append((shape, dtype))
    n_params = len(in_names)
    all_names = in_names + out_names
    if part_name is not None:
        all_names = all_names + [part_name]
    donate = tuple(range(n_params, n_params + len(out_names)))

    def _body(*args):
        operands = list(args)
        if part_name is not None:
            operands.append(bass2jax.partition_id_tensor())
        outs = bass2jax._bass_exec_p.bind(
            *operands,
            out_avals=tuple(out_avals),
            in_names=tuple(all_names),
            out_names=tuple(out_names),
            lowering_input_output_aliases=(),
            sim_require_finite=True,
            sim_require_nnan=True,
            nc=nc,
        )
        return tuple(outs)

    devices = jax.devices()[:N_CORES]
    mesh = Mesh(np.asarray(devices), ("core",))
    in_specs = (PartitionSpec("core"),) * (n_params + len(out_names))
    out_specs = (PartitionSpec("core"),) * len(out_names)
    sharded = jax.jit(
        shard_map(_body, mesh=mesh, in_specs=in_specs, out_specs=out_specs,
                  check_rep=False),
        donate_argnums=donate, keep_unused=True)

    def run(in_maps):
        concat_in = [
            np.concatenate([np.asarray(m[name]) for m in in_maps], axis=0)
            for name in in_names
        ]
        zeros = [np.zeros((N_CORES * s[0], *s[1:]), d) for s, d in zero_shapes]
        outs = sharded(*concat_in, *zeros)
        res = []
        for c in range(N_CORES):
            res.append({
                name: np.asarray(outs[i]).reshape(
                    N_CORES, *out_avals[i].shape)[c]
                for i, name in enumerate(out_names)
            })
        return res

    return run


def kernel(**inputs):
    shared, flags = _prep_shared(inputs)
    key = (flags["use_b1"], flags["use_b2"], flags["use_bqk"])
    in_maps = [_prep_core(inputs, c, shared) for c in range(N_CORES)]
    if key not in _RUNNERS:
        _RUNNERS[key] = _make_runner(key)
    res = _RUNNERS[key](in_maps)
    h2 = np.concatenate([res[c]["hidden2"] for c in range(N_CORES)], axis=0)
    at = np.concatenate([res[c]["attn"] for c in range(N_CORES)], axis=0)
    return (np.asarray(h2, np.float32), np.asarray(at, np.float32))
